# revision 19
# baseline (speedup 1.0000x reference)
"""Trainium2 Bass kernel: gather-rate-scatter metabolite update (one Euler
kinetics step) for B=262144 cells, data-parallel across 8 NeuronCores.

Math (per cell batch):
    enzyme = gene_expr @ G                      [B, 64]
    rates  = kcat * sigmoid(enzyme) * exp(log(conc+eps) @ max(-S,0))
    out    = max(conc + DT * (rates @ S.T), 0)  [B, 114]

Strategy
--------
- Pure data parallelism: B sharded across 8 cores; S/G/kcat replicated.
- Host-side layout prep: per-core shards are transposed so features sit on
  SBUF partitions. The small contractions then run natively on the tensor
  engine with zero on-chip transposes and fully contiguous DMA.
- Graph sparsity pruning (computed from S/G at run time): only genes that
  catalyze some reaction (~98/212), metabolites that are substrates (~74/114)
  and metabolites touched by the scatter (~100/114) are shipped to the
  device. Unused rows contribute exactly nothing.
- S has exactly two -1 entries per reaction, so the mass-action substrate
  term exp(log(conc+eps) @ s_neg) is conc[i1]*conc[i2]: both factors come
  from one-hot gather matmuls and one vector multiply — no Ln/Exp, so the
  scalar engine only runs Sigmoid/Copy out of a single activation-table set
  (avoids ~1.3us table reloads per function switch).
- kcat and DT fold into the scatter weights: delta = (DT*S*kcat).T @ rates.
- PSUM bank pairing: two 512-cell chunks share each 128-partition PSUM bank
  (chunk A on partitions 0:64, chunk B on 64:128), so every post-matmul
  vector/scalar op processes two chunks per instruction.
- The device ships the fp16 delta (DT * d_conc, magnitude ~0.1); the final
  out = relu(conc + delta) runs on the host in fp32 during unsharding. This
  keeps conc's full fp32 precision in the dominant term (strictly more
  accurate than an fp16 output) and halves the store traffic.
- Engine balance under the HBM roofline: loads issue from SP HWDGE, stores
  from Pool SWDGE (no head-of-line blocking of loads), the delta copy
  alternates DVE/ACT.
"""

import threading

import numpy as np

N_METS = 114
N_RXNS = 64
N_GENES = 212
B = 262144
N_CORES = 8
BS = B // N_CORES      # 32768 cells per core
CT = 2048              # cells per SBUF tile (DMA granularity)
NC = 512               # cells per PSUM chunk (half a paired fp32 bank)
DT = 0.01

_lock = threading.Lock()
_cached = {}


def _build_program(ng, ns, nt, ct=CT, io_bufs=4, mid_bufs=4, pe_bufs=2, pd_bufs=2, g2_dve_every=0):
    """ng = #used genes, ns = #substrate mets, nt = #scatter-touched mets."""
    import concourse.mybir as mybir
    import concourse.tile as tile
    from concourse import bacc

    f32 = mybir.dt.float32
    f16 = mybir.dt.float16
    AF = mybir.ActivationFunctionType

    # gene rows are loaded in <=128-partition groups (ng=98 here -> one)
    g_groups = [(g0, min(g0 + 128, ng)) for g0 in range(0, ng, 128)]

    nc = bacc.Bacc(
        "TRN2", target_bir_lowering=False, debug=False, num_devices=N_CORES
    )
    d_conc = nc.dram_tensor("conc_t", [ns, BS], f16, kind="ExternalInput").ap()
    d_gene = [
        nc.dram_tensor(f"gene_{i}", [g1 - g0, BS], f16, kind="ExternalInput").ap()
        for i, (g0, g1) in enumerate(g_groups)
    ]
    d_G = [
        nc.dram_tensor(f"g_map_{i}", [g1 - g0, N_RXNS], f16, kind="ExternalInput").ap()
        for i, (g0, g1) in enumerate(g_groups)
    ]
    # one-hot substrate selectors over the pruned met axis: col j ->
    # substrate1 of rxn j, col 64+j -> substrate2 of rxn j
    d_psel = nc.dram_tensor("p_sel", [ns, 2 * N_RXNS], f16, kind="ExternalInput").ap()
    # scatter weights with DT and kcat folded, over touched mets; duplicated
    # on both partition halves so mm_d can consume rates at base 0 or 64
    d_sdt = nc.dram_tensor("s_dtk", [2 * N_RXNS, nt], f16, kind="ExternalInput").ap()
    d_out = nc.dram_tensor("out_t", [nt, BS], f16, kind="ExternalOutput").ap()

    with tile.TileContext(nc) as tc:
        with (
            tc.tile_pool(name="consts", bufs=1) as consts,
            tc.tile_pool(name="io", bufs=io_bufs) as io,
            tc.tile_pool(name="mid", bufs=mid_bufs) as mid,
            tc.tile_pool(name="ps_e", bufs=pe_bufs, space="PSUM") as ps_e,
            tc.tile_pool(name="ps_ga", bufs=2, space="PSUM") as ps_ga,
            tc.tile_pool(name="ps_gb", bufs=2, space="PSUM") as ps_gb,
            tc.tile_pool(name="ps_d", bufs=pd_bufs, space="PSUM") as ps_d,
        ):
            c_G = []
            for i, (g0, g1) in enumerate(g_groups):
                t = consts.tile([g1 - g0, N_RXNS], f16, name=f"c_G{i}")
                nc.sync.dma_start(out=t, in_=d_G[i])
                c_G.append(t)
            c_psel = consts.tile([ns, 2 * N_RXNS], f16)
            nc.sync.dma_start(out=c_psel, in_=d_psel)
            c_sdt = consts.tile([2 * N_RXNS, nt], f16)
            nc.sync.dma_start(out=c_sdt, in_=d_sdt)

            for it in range(BS // ct):
                sl = slice(it * ct, (it + 1) * ct)
                t_conc = io.tile([ns, ct], f16, tag="conc")
                nc.sync.dma_start(out=t_conc, in_=d_conc[:, sl])
                t_gene = []
                for i, (g0, g1) in enumerate(g_groups):
                    t = io.tile([g1 - g0, ct], f16, tag=f"gene{i}", name=f"t_gene{i}")
                    nc.sync.dma_start(out=t, in_=d_gene[i][:, sl])
                    t_gene.append(t)
                t_out = io.tile([nt, ct], f16, tag="out")

                for ip in range(ct // (2 * NC)):
                    cs0 = slice((2 * ip) * NC, (2 * ip + 1) * NC)
                    cs1 = slice((2 * ip + 1) * NC, (2 * ip + 2) * NC)
                    # two 512-cell chunks share each PSUM bank (partitions
                    # 0:64 and 64:128) so post-matmul ops cover both at once
                    p_e = ps_e.tile([2 * N_RXNS, NC], f32, tag="pe")
                    p_ga = ps_ga.tile([2 * N_RXNS, NC], f32, tag="pga")
                    p_gb = ps_gb.tile([2 * N_RXNS, NC], f32, tag="pgb")
                    for s, cs in enumerate((cs0, cs1)):
                        half = slice(s * N_RXNS, (s + 1) * N_RXNS)
                        for i, t in enumerate(t_gene):
                            nc.tensor.matmul(
                                p_e[half], c_G[i], t[:, cs],
                                start=(i == 0), stop=(i == len(t_gene) - 1),
                            )
                        nc.tensor.matmul(
                            p_ga[half], c_psel[:, :N_RXNS], t_conc[:, cs],
                            start=True, stop=True,
                        )
                        nc.tensor.matmul(
                            p_gb[half], c_psel[:, N_RXNS:], t_conc[:, cs],
                            start=True, stop=True,
                        )
                    t_sig = mid.tile([2 * N_RXNS, NC], f16, tag="sig")
                    nc.scalar.activation(t_sig, p_e, AF.Sigmoid)
                    t_g2 = mid.tile([2 * N_RXNS, NC], f16, tag="g2")
                    if g2_dve_every and ip % g2_dve_every == 0:
                        nc.vector.tensor_copy(t_g2, p_gb)
                    else:
                        nc.scalar.activation(t_g2, p_gb, AF.Copy)
                    t_prod = mid.tile([2 * N_RXNS, NC], f16, tag="prod")
                    nc.vector.tensor_mul(t_prod, p_ga, t_g2)
                    t_rates = mid.tile([2 * N_RXNS, NC], f16, tag="rates")
                    nc.vector.tensor_mul(t_rates, t_sig, t_prod)
                    for s, cs in enumerate((cs0, cs1)):
                        half = slice(s * N_RXNS, (s + 1) * N_RXNS)
                        p_d = ps_d.tile([nt, NC], f32, tag="pd")
                        nc.tensor.matmul(
                            p_d, c_sdt[half], t_rates[half], start=True, stop=True
                        )
                        # fp16 delta out; host adds conc and clamps. The copy
                        # alternates DVE/ACT to balance both engines.
                        if s == 0:
                            nc.vector.tensor_copy(t_out[:, cs], p_d)
                        else:
                            nc.scalar.activation(t_out[:, cs], p_d, AF.Copy)

                # store from the Pool engine (SWDGE) so a store waiting on the
                # compute tail never head-of-line-blocks the SP loads
                nc.gpsimd.dma_start(out=d_out[:, sl], in_=t_out)

    nc.compile()
    return nc


def _get_program(ng, ns, nt):
    key = (ng, ns, nt)
    with _lock:
        if key not in _cached:
            _cached[key] = _build_program(ng, ns, nt)
        return _cached[key]


def _graph_consts(S, G, kcat):
    """Analyze the (replicated, tiny) graph tensors and build device consts."""
    used_genes = np.where(G.any(axis=1))[0]
    used_subs = np.where((S < 0).any(axis=1))[0]
    touched = np.where((S != 0).any(axis=1))[0]
    ng, ns = len(used_genes), len(used_subs)

    sub_pos = {m: i for i, m in enumerate(used_subs)}
    p_sel = np.zeros((ns, 2 * N_RXNS), dtype=np.float16)
    for j in range(N_RXNS):
        subs = np.where(S[:, j] < 0)[0]
        assert len(subs) == 2, f"reaction {j} has {len(subs)} substrates"
        p_sel[sub_pos[subs[0]], j] = 1.0
        p_sel[sub_pos[subs[1]], N_RXNS + j] = 1.0

    sdtk = (DT * S[touched] * kcat[None, :]).T.astype(np.float16)  # [64, nt]
    g_used = G[used_genes].astype(np.float16)                      # [ng, 64]

    consts = {"p_sel": p_sel, "s_dtk": np.ascontiguousarray(np.vstack([sdtk, sdtk]))}
    for i, g0 in enumerate(range(0, ng, 128)):
        consts[f"g_map_{i}"] = np.ascontiguousarray(g_used[g0 : g0 + 128])
    return consts, used_genes, used_subs, touched


def kernel(conc, gene_expr, S, G, kcat):
    from concourse.bass_utils import run_bass_kernel_spmd

    conc = np.asarray(conc, dtype=np.float32)
    gene_expr = np.asarray(gene_expr, dtype=np.float32)
    S = np.asarray(S, dtype=np.float32)
    G = np.asarray(G, dtype=np.float32)
    kcat = np.asarray(kcat, dtype=np.float32)

    consts, used_genes, used_subs, touched = _graph_consts(S, G, kcat)
    ng, ns, nt = len(used_genes), len(used_subs), len(touched)
    nc = _get_program(ng, ns, nt)

    in_maps = []
    for c in range(N_CORES):
        rows = slice(c * BS, (c + 1) * BS)
        gene_t = gene_expr[rows, :].T[used_genes].astype(np.float16)  # [ng, BS]
        m = {
            "conc_t": np.ascontiguousarray(
                conc[rows, :].T[used_subs].astype(np.float16)
            ),
            **consts,
        }
        for i, g0 in enumerate(range(0, ng, 128)):
            m[f"gene_{i}"] = np.ascontiguousarray(gene_t[g0 : g0 + 128])
        in_maps.append(m)

    res = run_bass_kernel_spmd(nc, in_maps, core_ids=list(range(N_CORES)))

    # device ships the fp16 delta DT*d_conc on touched mets; finish
    # out = relu(conc + delta) in fp32 on the host (keeps conc's full
    # precision in the dominant term). Untouched mets keep delta == 0.
    out = conc.copy()
    for c in range(N_CORES):
        rows = slice(c * BS, (c + 1) * BS)
        delta = res.results[c]["out_t"].T.astype(np.float32)  # [BS, nt]
        out[rows, touched] += delta
    np.maximum(out, 0.0, out=out)
    return out


# revision 20
# speedup vs baseline: 1.0033x; 1.0033x over previous
"""Trainium2 Bass kernel: gather-rate-scatter metabolite update (one Euler
kinetics step) for B=262144 cells, data-parallel across 8 NeuronCores.

Math (per cell batch):
    enzyme = gene_expr @ G                      [B, 64]
    rates  = kcat * sigmoid(enzyme) * exp(log(conc+eps) @ max(-S,0))
    out    = max(conc + DT * (rates @ S.T), 0)  [B, 114]

Strategy
--------
- Pure data parallelism: B sharded across 8 cores; S/G/kcat replicated.
- Host-side layout prep: per-core shards are transposed so features sit on
  SBUF partitions. The small contractions then run natively on the tensor
  engine with zero on-chip transposes and fully contiguous DMA.
- Graph sparsity pruning (computed from S/G at run time): only genes that
  catalyze some reaction (~98/212), metabolites that are substrates (~74/114)
  and metabolites touched by the scatter (~100/114) are shipped to the
  device. Unused rows contribute exactly nothing.
- S has exactly two -1 entries per reaction, so the mass-action substrate
  term exp(log(conc+eps) @ s_neg) is conc[i1]*conc[i2]: both factors come
  from one-hot gather matmuls and one vector multiply — no Ln/Exp, so the
  scalar engine only runs Sigmoid/Copy out of a single activation-table set
  (avoids ~1.3us table reloads per function switch).
- kcat and DT fold into the scatter weights: delta = (DT*S*kcat).T @ rates.
- PSUM bank pairing: two 512-cell chunks share each 128-partition PSUM bank
  (chunk A on partitions 0:64, chunk B on 64:128), so every post-matmul
  vector/scalar op processes two chunks per instruction.
- The device ships the fp16 delta (DT * d_conc, magnitude ~0.1); the final
  out = relu(conc + delta) runs on the host in fp32 during unsharding. This
  keeps conc's full fp32 precision in the dominant term (strictly more
  accurate than an fp16 output) and halves the store traffic.
- Engine balance under the HBM roofline: loads issue from SP HWDGE, stores
  from Pool SWDGE (no head-of-line blocking of loads), the delta copy
  alternates DVE/ACT.
"""

import threading

import numpy as np

N_METS = 114
N_RXNS = 64
N_GENES = 212
B = 262144
N_CORES = 8
BS = B // N_CORES      # 32768 cells per core
CT = 2048              # cells per SBUF tile (DMA granularity)
NC = 512               # cells per PSUM chunk (half a paired fp32 bank)
DT = 0.01

_lock = threading.Lock()
_cached = {}


def _build_program(ng, ns, nt, ct=CT, io_bufs=4, mid_bufs=4, pe_bufs=1, pd_bufs=3, g2_dve_every=0):
    """ng = #used genes, ns = #substrate mets, nt = #scatter-touched mets."""
    import concourse.mybir as mybir
    import concourse.tile as tile
    from concourse import bacc

    f32 = mybir.dt.float32
    f16 = mybir.dt.float16
    AF = mybir.ActivationFunctionType

    # gene rows are loaded in <=128-partition groups (ng=98 here -> one)
    g_groups = [(g0, min(g0 + 128, ng)) for g0 in range(0, ng, 128)]

    nc = bacc.Bacc(
        "TRN2", target_bir_lowering=False, debug=False, num_devices=N_CORES
    )
    d_conc = nc.dram_tensor("conc_t", [ns, BS], f16, kind="ExternalInput").ap()
    d_gene = [
        nc.dram_tensor(f"gene_{i}", [g1 - g0, BS], f16, kind="ExternalInput").ap()
        for i, (g0, g1) in enumerate(g_groups)
    ]
    d_G = [
        nc.dram_tensor(f"g_map_{i}", [g1 - g0, N_RXNS], f16, kind="ExternalInput").ap()
        for i, (g0, g1) in enumerate(g_groups)
    ]
    # one-hot substrate selectors over the pruned met axis: col j ->
    # substrate1 of rxn j, col 64+j -> substrate2 of rxn j
    d_psel = nc.dram_tensor("p_sel", [ns, 2 * N_RXNS], f16, kind="ExternalInput").ap()
    # scatter weights with DT and kcat folded, over touched mets; duplicated
    # on both partition halves so mm_d can consume rates at base 0 or 64
    d_sdt = nc.dram_tensor("s_dtk", [2 * N_RXNS, nt], f16, kind="ExternalInput").ap()
    d_out = nc.dram_tensor("out_t", [nt, BS], f16, kind="ExternalOutput").ap()

    with tile.TileContext(nc) as tc:
        with (
            tc.tile_pool(name="consts", bufs=1) as consts,
            tc.tile_pool(name="io", bufs=io_bufs) as io,
            tc.tile_pool(name="mid", bufs=mid_bufs) as mid,
            tc.tile_pool(name="ps_e", bufs=pe_bufs, space="PSUM") as ps_e,
            tc.tile_pool(name="ps_ga", bufs=2, space="PSUM") as ps_ga,
            tc.tile_pool(name="ps_gb", bufs=2, space="PSUM") as ps_gb,
            tc.tile_pool(name="ps_d", bufs=pd_bufs, space="PSUM") as ps_d,
        ):
            c_G = []
            for i, (g0, g1) in enumerate(g_groups):
                t = consts.tile([g1 - g0, N_RXNS], f16, name=f"c_G{i}")
                nc.sync.dma_start(out=t, in_=d_G[i])
                c_G.append(t)
            c_psel = consts.tile([ns, 2 * N_RXNS], f16)
            nc.sync.dma_start(out=c_psel, in_=d_psel)
            c_sdt = consts.tile([2 * N_RXNS, nt], f16)
            nc.sync.dma_start(out=c_sdt, in_=d_sdt)

            for it in range(BS // ct):
                sl = slice(it * ct, (it + 1) * ct)
                t_conc = io.tile([ns, ct], f16, tag="conc")
                nc.sync.dma_start(out=t_conc, in_=d_conc[:, sl])
                t_gene = []
                for i, (g0, g1) in enumerate(g_groups):
                    t = io.tile([g1 - g0, ct], f16, tag=f"gene{i}", name=f"t_gene{i}")
                    nc.sync.dma_start(out=t, in_=d_gene[i][:, sl])
                    t_gene.append(t)
                t_out = io.tile([nt, ct], f16, tag="out")

                for ip in range(ct // (2 * NC)):
                    cs0 = slice((2 * ip) * NC, (2 * ip + 1) * NC)
                    cs1 = slice((2 * ip + 1) * NC, (2 * ip + 2) * NC)
                    # two 512-cell chunks share each PSUM bank (partitions
                    # 0:64 and 64:128) so post-matmul ops cover both at once
                    p_e = ps_e.tile([2 * N_RXNS, NC], f32, tag="pe")
                    p_ga = ps_ga.tile([2 * N_RXNS, NC], f32, tag="pga")
                    p_gb = ps_gb.tile([2 * N_RXNS, NC], f32, tag="pgb")
                    for s, cs in enumerate((cs0, cs1)):
                        half = slice(s * N_RXNS, (s + 1) * N_RXNS)
                        for i, t in enumerate(t_gene):
                            nc.tensor.matmul(
                                p_e[half], c_G[i], t[:, cs],
                                start=(i == 0), stop=(i == len(t_gene) - 1),
                            )
                        nc.tensor.matmul(
                            p_ga[half], c_psel[:, :N_RXNS], t_conc[:, cs],
                            start=True, stop=True,
                        )
                        nc.tensor.matmul(
                            p_gb[half], c_psel[:, N_RXNS:], t_conc[:, cs],
                            start=True, stop=True,
                        )
                    t_sig = mid.tile([2 * N_RXNS, NC], f16, tag="sig")
                    nc.scalar.activation(t_sig, p_e, AF.Sigmoid)
                    t_g2 = mid.tile([2 * N_RXNS, NC], f16, tag="g2")
                    if g2_dve_every and ip % g2_dve_every == 0:
                        nc.vector.tensor_copy(t_g2, p_gb)
                    else:
                        nc.scalar.activation(t_g2, p_gb, AF.Copy)
                    t_prod = mid.tile([2 * N_RXNS, NC], f16, tag="prod")
                    nc.vector.tensor_mul(t_prod, p_ga, t_g2)
                    t_rates = mid.tile([2 * N_RXNS, NC], f16, tag="rates")
                    nc.vector.tensor_mul(t_rates, t_sig, t_prod)
                    for s, cs in enumerate((cs0, cs1)):
                        half = slice(s * N_RXNS, (s + 1) * N_RXNS)
                        p_d = ps_d.tile([nt, NC], f32, tag="pd")
                        nc.tensor.matmul(
                            p_d, c_sdt[half], t_rates[half], start=True, stop=True
                        )
                        # fp16 delta out; host adds conc and clamps. The copy
                        # alternates DVE/ACT to balance both engines.
                        if s == 0:
                            nc.vector.tensor_copy(t_out[:, cs], p_d)
                        else:
                            nc.scalar.activation(t_out[:, cs], p_d, AF.Copy)

                # store from the Pool engine (SWDGE) so a store waiting on the
                # compute tail never head-of-line-blocks the SP loads
                nc.gpsimd.dma_start(out=d_out[:, sl], in_=t_out)

    nc.compile()
    return nc


def _get_program(ng, ns, nt):
    key = (ng, ns, nt)
    with _lock:
        if key not in _cached:
            _cached[key] = _build_program(ng, ns, nt)
        return _cached[key]


def _graph_consts(S, G, kcat):
    """Analyze the (replicated, tiny) graph tensors and build device consts."""
    used_genes = np.where(G.any(axis=1))[0]
    used_subs = np.where((S < 0).any(axis=1))[0]
    touched = np.where((S != 0).any(axis=1))[0]
    ng, ns = len(used_genes), len(used_subs)

    sub_pos = {m: i for i, m in enumerate(used_subs)}
    p_sel = np.zeros((ns, 2 * N_RXNS), dtype=np.float16)
    for j in range(N_RXNS):
        subs = np.where(S[:, j] < 0)[0]
        assert len(subs) == 2, f"reaction {j} has {len(subs)} substrates"
        p_sel[sub_pos[subs[0]], j] = 1.0
        p_sel[sub_pos[subs[1]], N_RXNS + j] = 1.0

    sdtk = (DT * S[touched] * kcat[None, :]).T.astype(np.float16)  # [64, nt]
    g_used = G[used_genes].astype(np.float16)                      # [ng, 64]

    consts = {"p_sel": p_sel, "s_dtk": np.ascontiguousarray(np.vstack([sdtk, sdtk]))}
    for i, g0 in enumerate(range(0, ng, 128)):
        consts[f"g_map_{i}"] = np.ascontiguousarray(g_used[g0 : g0 + 128])
    return consts, used_genes, used_subs, touched


def kernel(conc, gene_expr, S, G, kcat):
    from concourse.bass_utils import run_bass_kernel_spmd

    conc = np.asarray(conc, dtype=np.float32)
    gene_expr = np.asarray(gene_expr, dtype=np.float32)
    S = np.asarray(S, dtype=np.float32)
    G = np.asarray(G, dtype=np.float32)
    kcat = np.asarray(kcat, dtype=np.float32)

    consts, used_genes, used_subs, touched = _graph_consts(S, G, kcat)
    ng, ns, nt = len(used_genes), len(used_subs), len(touched)
    nc = _get_program(ng, ns, nt)

    in_maps = []
    for c in range(N_CORES):
        rows = slice(c * BS, (c + 1) * BS)
        gene_t = gene_expr[rows, :].T[used_genes].astype(np.float16)  # [ng, BS]
        m = {
            "conc_t": np.ascontiguousarray(
                conc[rows, :].T[used_subs].astype(np.float16)
            ),
            **consts,
        }
        for i, g0 in enumerate(range(0, ng, 128)):
            m[f"gene_{i}"] = np.ascontiguousarray(gene_t[g0 : g0 + 128])
        in_maps.append(m)

    res = run_bass_kernel_spmd(nc, in_maps, core_ids=list(range(N_CORES)))

    # device ships the fp16 delta DT*d_conc on touched mets; finish
    # out = relu(conc + delta) in fp32 on the host (keeps conc's full
    # precision in the dominant term). Untouched mets keep delta == 0.
    out = conc.copy()
    for c in range(N_CORES):
        rows = slice(c * BS, (c + 1) * BS)
        delta = res.results[c]["out_t"].T.astype(np.float32)  # [BS, nt]
        out[rows, touched] += delta
    np.maximum(out, 0.0, out=out)
    return out
